# revision 1
# baseline (speedup 1.0000x reference)
"""Batched Sinkhorn-divergence loss (geomloss-style) distributed over 8 NeuronCores.

Data-parallel sharding per the problem's sharding hint: the graph/batch axis
G=64 is split across 8 devices (8 graphs per core). Each device computes its
local Sinkhorn divergences (log-domain, 20 iterations, blur=0.05, p=2) and the
partial sums are combined into the global mean on the host.

Self-contained: shapes/constants hardcoded for x, target: [64, 1024, 16] f32.
"""

import numpy as np
import jax
import jax.numpy as jnp

P = 2
BLUR = 0.05
EPS = BLUR ** P
N_ITERS = 20

G, N, D = 64, 1024, 16
N_CORES = 8


def _cost(x, y):
    x2 = jnp.sum(x * x, axis=-1)
    y2 = jnp.sum(y * y, axis=-1)
    xy = x @ y.T
    C = 0.5 * (x2[:, None] + y2[None, :] - 2.0 * xy)
    return jnp.maximum(C, 0.0)


def _ot_eps(x, y):
    C = _cost(x, y)
    n, m = C.shape
    loga = -np.log(n).astype(np.float32)
    logb = -np.log(m).astype(np.float32)
    Ce = C / EPS

    def step(g, _):
        f = -EPS * jax.nn.logsumexp(g[None, :] / EPS - Ce + logb, axis=1)
        g_new = -EPS * jax.nn.logsumexp(f[:, None] / EPS - Ce + loga, axis=0)
        return g_new, None

    g0 = jnp.zeros((m,), dtype=x.dtype)
    g, _ = jax.lax.scan(step, g0, None, length=N_ITERS)
    f = -EPS * jax.nn.logsumexp(g[None, :] / EPS - Ce + logb, axis=1)
    return f.mean() + g.mean()


def _sinkhorn_divergence(x, y):
    return _ot_eps(x, y) - 0.5 * _ot_eps(x, x) - 0.5 * _ot_eps(y, y)


def _shard_loss_sum(xs, ys):
    # xs, ys: [G/N_CORES, N, D] — sum (not mean) of local divergences
    losses = jax.vmap(_sinkhorn_divergence)(xs, ys)
    return jnp.sum(losses)


_pmapped = None


def _get_pmapped():
    global _pmapped
    if _pmapped is None:
        _pmapped = jax.pmap(_shard_loss_sum)
    return _pmapped


def kernel(x: np.ndarray, target: np.ndarray) -> np.ndarray:
    x = np.asarray(x, dtype=np.float32).reshape(G, N, D)
    target = np.asarray(target, dtype=np.float32).reshape(G, N, D)

    per = G // N_CORES
    xs = x.reshape(N_CORES, per, N, D)
    ys = target.reshape(N_CORES, per, N, D)

    try:
        devs = jax.devices()
        if len(devs) >= N_CORES:
            partial = _get_pmapped()(xs, ys)  # [N_CORES]
            total = np.asarray(partial, dtype=np.float64).sum()
        else:
            raise RuntimeError("fewer than 8 devices")
    except Exception:
        # Fallback: single-device execution (still correct)
        f = jax.jit(_shard_loss_sum)
        total = 0.0
        for c in range(N_CORES):
            total += float(f(xs[c], ys[c]))

    out = np.float32(total / G)
    return np.asarray(out, dtype=np.float32)

